# revision 8
# baseline (speedup 1.0000x reference)
import numpy as np
import jax
import jax.numpy as jnp

# nn_Backflow: 3 rounds of GNN message passing over 32 electrons.
# Data parallel: shard the 2048-walker batch across 8 NeuronCores;
# tiny MLP params are replicated. No cross-core communication.

CUTOFF = 10.0
BASIS_DIM = 32
N_ELEC = 32
N_CORES = 8
BATCH = 2048
LOG2 = float(np.log(2.0))


def _offdiag_indices(n):
    idx = np.arange(n)
    ii = np.repeat(idx, n - 1)
    jj = np.concatenate([np.delete(idx, i) for i in range(n)])
    return ii, jj


_II, _JJ = _offdiag_indices(N_ELEC)


def _forward(rs, params):
    # rs: [Bs, 32, 3] one shard of walkers
    delta = 1.0 / (2 * BASIS_DIM)
    qs = jnp.linspace(delta, 1 - delta, BASIS_DIM)
    mus = CUTOFF * qs**2
    sigmas = (1 + CUTOFF * qs) / 7.0

    B, n, _ = rs.shape
    xs = rs
    for (W1, b1), (W2, b2), (W3,) in params:
        diffs = xs[:, _JJ] - xs[:, _II]                    # [B, E, 3]
        d2 = jnp.sum(diffs * diffs, axis=-1)               # [B, E]
        dists = jnp.sqrt(d2)
        dr = dists / CUTOFF
        # p(u) = 1-10u^3+15u^4-6u^5 is monotone decreasing with p(1)=0, so
        # where(u>1, 0, p(u)) == relu(p(u)) exactly — avoids a select fusion
        # the neuron compiler cannot lower.
        u3 = dr * dr * dr
        env = jax.nn.relu(1 + u3 * (-10.0 + dr * (15.0 - 6.0 * dr)))
        basis = env[..., None] * jnp.exp(-((dists[..., None] - mus) ** 2) / sigmas**2)
        # softplus(x) = log(1+e^x); pre-activations here are bounded (|x| < ~25)
        # so the naive form is fp32-safe and avoids Softplus activation-table
        # fusions the neuron compiler cannot lower.
        # ssp(x) = log((1+e^x)/2) = log(0.5 + 0.5*e^x); written directly so the
        # tensorizer doesn't pattern-match a Softplus (which fails to lower).
        h = jnp.log(0.5 + 0.5 * jnp.exp(basis @ W1.T + b1))    # [B, E, 10]
        h = jnp.log(0.5 + 0.5 * jnp.exp(h @ W2.T + b2))        # [B, E, 3]
        w = h @ W3.T                                       # [B, E, 1]
        zs = (w * diffs).reshape(B, n, n - 1, 3).sum(axis=2)
        xs = xs + zs
    return xs


_PMAPPED = None


def _get_pmapped():
    global _PMAPPED
    if _PMAPPED is None:
        _PMAPPED = jax.pmap(_forward, in_axes=(0, None),
                            devices=jax.devices()[:N_CORES])
    return _PMAPPED


def kernel(rs, params):
    rs = np.asarray(rs, dtype=np.float32)
    assert rs.shape == (BATCH, N_ELEC, 3)
    params = jax.tree_util.tree_map(lambda a: np.asarray(a, dtype=np.float32), params)
    rs_sharded = rs.reshape(N_CORES, BATCH // N_CORES, N_ELEC, 3)
    out = _get_pmapped()(rs_sharded, params)
    return np.asarray(out).reshape(BATCH, N_ELEC, 3).astype(np.float32)


# revision 11
# speedup vs baseline: 1.0056x; 1.0056x over previous
import numpy as np
import jax
import jax.numpy as jnp

# nn_Backflow: 3 rounds of GNN message passing over 32 electrons.
# Data parallel: shard the 2048-walker batch across 8 NeuronCores;
# tiny MLP params are replicated. No cross-core communication.

CUTOFF = 10.0
BASIS_DIM = 32
N_ELEC = 32
N_CORES = 8
BATCH = 2048
LOG2 = float(np.log(2.0))


def _offdiag_indices(n):
    idx = np.arange(n)
    ii = np.repeat(idx, n - 1)
    jj = np.concatenate([np.delete(idx, i) for i in range(n)])
    return ii, jj


_II, _JJ = _offdiag_indices(N_ELEC)


def _forward(rs, params):
    # rs: [Bs, 32, 3] one shard of walkers
    delta = 1.0 / (2 * BASIS_DIM)
    qs = jnp.linspace(delta, 1 - delta, BASIS_DIM)
    mus = CUTOFF * qs**2
    sigmas = (1 + CUTOFF * qs) / 7.0

    B, n, _ = rs.shape
    xs = rs
    for (W1, b1), (W2, b2), (W3,) in params:
        diffs = xs[:, _JJ] - xs[:, _II]                    # [B, E, 3]
        d2 = jnp.sum(diffs * diffs, axis=-1)               # [B, E]
        dists = jnp.sqrt(d2)
        dr = dists / CUTOFF
        # p(u) = 1-10u^3+15u^4-6u^5 is monotone decreasing with p(1)=0, so
        # where(u>1, 0, p(u)) == relu(p(u)) exactly — avoids a select fusion
        # the neuron compiler cannot lower.
        u3 = dr * dr * dr
        env = jax.nn.relu(1 + u3 * (-10.0 + dr * (15.0 - 6.0 * dr)))
        basis = env[..., None] * jnp.exp(-((dists[..., None] - mus) ** 2) / sigmas**2)
        # softplus(x) = log(1+e^x); pre-activations here are bounded (|x| < ~25)
        # so the naive form is fp32-safe and avoids Softplus activation-table
        # fusions the neuron compiler cannot lower.
        # ssp(x) = log((1+e^x)/2) = log(0.5 + 0.5*e^x); written directly so the
        # tensorizer doesn't pattern-match a Softplus (which fails to lower).
        h = jnp.log(0.5 + 0.5 * jnp.exp(basis @ W1.T + b1))    # [B, E, 10]
        h = jnp.log(0.5 + 0.5 * jnp.exp(h @ W2.T + b2))        # [B, E, 3]
        w = h @ W3.T                                       # [B, E, 1]
        zs = (w * diffs).reshape(B, n, n - 1, 3).sum(axis=2)
        xs = xs + zs
    return xs


_PMAPPED = None


def _get_pmapped():
    global _PMAPPED
    if _PMAPPED is None:
        _PMAPPED = jax.pmap(_forward, in_axes=(0, None),
                            devices=jax.devices()[:N_CORES])
    return _PMAPPED


def kernel(rs, params):
    rs = np.asarray(rs, dtype=np.float32)
    assert rs.shape == (BATCH, N_ELEC, 3)
    params = jax.tree_util.tree_map(lambda a: np.asarray(a, dtype=np.float32), params)
    rs_sharded = rs.reshape(N_CORES, BATCH // N_CORES, N_ELEC, 3)
    out = _get_pmapped()(rs_sharded, params)
    return np.asarray(out).reshape(BATCH, N_ELEC, 3).astype(np.float32)


# revision 12
# speedup vs baseline: 1.1216x; 1.1153x over previous
import numpy as np
import jax
import jax.numpy as jnp

# nn_Backflow: 3 rounds of GNN message passing over 32 electrons.
# Data parallel: shard the 2048-walker batch across 8 NeuronCores;
# tiny MLP params are replicated. No cross-core communication.

CUTOFF = 10.0
BASIS_DIM = 32
N_ELEC = 32
N_CORES = 8
BATCH = 2048
LOG2 = float(np.log(2.0))


def _offdiag_indices(n):
    idx = np.arange(n)
    ii = np.repeat(idx, n - 1)
    jj = np.concatenate([np.delete(idx, i) for i in range(n)])
    return ii, jj


_II, _JJ = _offdiag_indices(N_ELEC)


def _forward(rs, params):
    # rs: [Bs, 32, 3] one shard of walkers
    delta = 1.0 / (2 * BASIS_DIM)
    qs = jnp.linspace(delta, 1 - delta, BASIS_DIM)
    mus = CUTOFF * qs**2
    sigmas = (1 + CUTOFF * qs) / 7.0

    B, n, _ = rs.shape
    xs = rs
    for (W1, b1), (W2, b2), (W3,) in params:
        # Edge list is i-major, so the ii side is a 31-fold repeat of xs:
        # broadcast it instead of gathering — halves gather traffic. All
        # downstream shapes stay [B, E, ...].
        xj = xs[:, _JJ].reshape(B, n, n - 1, 3)            # [B, n, n-1, 3]
        diffs = (xj - xs[:, :, None, :]).reshape(B, n * (n - 1), 3)
        d2 = jnp.sum(diffs * diffs, axis=-1)               # [B, E]
        dists = jnp.sqrt(d2)
        dr = dists / CUTOFF
        # p(u) = 1-10u^3+15u^4-6u^5 is monotone decreasing with p(1)=0, so
        # where(u>1, 0, p(u)) == relu(p(u)) exactly — avoids a select fusion
        # the neuron compiler cannot lower.
        u3 = dr * dr * dr
        env = jax.nn.relu(1 + u3 * (-10.0 + dr * (15.0 - 6.0 * dr)))
        basis = env[..., None] * jnp.exp(-((dists[..., None] - mus) ** 2) / sigmas**2)
        # softplus(x) = log(1+e^x); pre-activations here are bounded (|x| < ~25)
        # so the naive form is fp32-safe and avoids Softplus activation-table
        # fusions the neuron compiler cannot lower.
        # ssp(x) = log((1+e^x)/2) = log(0.5 + 0.5*e^x); written directly so the
        # tensorizer doesn't pattern-match a Softplus (which fails to lower).
        h = jnp.log(0.5 + 0.5 * jnp.exp(basis @ W1.T + b1))    # [B, E, 10]
        h = jnp.log(0.5 + 0.5 * jnp.exp(h @ W2.T + b2))        # [B, E, 3]
        w = h @ W3.T                                       # [B, E, 1]
        zs = (w * diffs).reshape(B, n, n - 1, 3).sum(axis=2)
        xs = xs + zs
    return xs


_PMAPPED = None


def _get_pmapped():
    global _PMAPPED
    if _PMAPPED is None:
        _PMAPPED = jax.pmap(_forward, in_axes=(0, None),
                            devices=jax.devices()[:N_CORES])
    return _PMAPPED


def kernel(rs, params):
    rs = np.asarray(rs, dtype=np.float32)
    assert rs.shape == (BATCH, N_ELEC, 3)
    params = jax.tree_util.tree_map(lambda a: np.asarray(a, dtype=np.float32), params)
    rs_sharded = rs.reshape(N_CORES, BATCH // N_CORES, N_ELEC, 3)
    out = _get_pmapped()(rs_sharded, params)
    return np.asarray(out).reshape(BATCH, N_ELEC, 3).astype(np.float32)
